# revision 25
# baseline (speedup 1.0000x reference)
"""Trainium2 Bass kernel: single-head causal attention (B=8, T=2048, D=1024, HS=64).

Sharding: data-parallel over batch B -- one batch element per NeuronCore (8 cores).
Host-side prep (part of sharding/layout): per-core x is passed transposed (d-major)
so the contraction dim lands on SBUF partitions; weights are packed/transposed
into a single [128, 1601] image so they arrive in one DMA.

Per-core device algorithm (v3 -- software-pipelined two blocks deep):
  x.T streamed in 8 full-T d-chunk slabs (4KB rows keep the DMA engines at
  full burst rate), alternating between the two HWDGE rings.
  [Q.T; K.T] (stacked on partitions) = [wq; wk].T-chunks @ x.T (PSUM-accumulated)
  V.T = wv.T-chunks @ x.T, PE-transposed to natural V [t, h] with an appended
        ones-column (row-sum trick).
  Attention in transposed layout: S.T[tk, tq] = K.T_chunk.T @ Q.T, exp on ScalarE
  (scale 1/sqrt(HS) fused, no max-subtraction -- scores are O(1) gaussian),
  causal via chunk skipping, triangular moving-range slicing, and a 0/1
  triangular-mask multiply on DVE restricted to the 128-wide diagonal sub-block.
  O.T_unnorm[h+1, tq] accumulates V'_chunk.T @ P.T over tk chunks; row HS is the
  softmax denominator. Final PE transpose to [tq, h+1], DVE reciprocal*mul,
  per-128-row DMA out.
  Pipeline: while block j's O matmuls and finalize run, the projections AND all
  S-matmuls/exps of block j+1 are woven between them, so ScalarE's exp stream
  runs a full block ahead and the PE never drains (keeps the p-state at max).
  Matmuls run in bf16 (full PE rate, half DMA bytes); exp/normalize in fp32.
"""
import os
import sys

for _p in ("/opt/trn_rl_repo", "/root/.axon_site/_ro/trn_rl_repo"):
    if _p not in sys.path and os.path.isdir(_p):
        sys.path.append(_p)

import numpy as np
import jax

try:
    jax.config.update("jax_compilation_cache_dir", "/tmp/jax_neff_cache")
    jax.config.update("jax_persistent_cache_min_compile_time_secs", 1.0)
    jax.config.update("jax_persistent_cache_min_entry_size_bytes", -1)
except Exception:
    pass

import concourse.mybir as mybir
import concourse.tile as tile
from concourse import bacc
from concourse.bass_utils import run_bass_kernel_spmd
from concourse.masks import make_identity

B, T, D, HS = 8, 2048, 1024, 64
NCORES = 8
QB = 512            # query block (free dim of S.T tiles / PSUM bank width)
KC = 128            # key chunk (partition dim of S.T tiles)
NQB = T // QB       # 4
NKC = T // KC       # 16
ND = D // 128       # 8 contraction chunks
WPK = ND * 2 * HS + ND * HS + HS + 1   # packed weight image columns (1601)

MM_MODE = os.environ.get("BASS_MM_MODE", "bf16")   # "f32" | "f32r" | "bf16"
FALLBACK_MODE = "f32"   # numerically safe mode if the fast mode misbehaves on HW

F32 = mybir.dt.float32
_MM_DTS = {"f32": F32, "f32r": mybir.dt.float32r, "bf16": mybir.dt.bfloat16}


def build(mode=None):
    MM = _MM_DTS[mode or MM_MODE]
    nc = bacc.Bacc(None)
    # x.T pre-packed host-side so each SBUF partition's data is one
    # contiguous DRAM run: xTp[p, dc*T + t] = x.T[dc*128 + p, t].  The
    # whole load is then 128 huge descriptors instead of 1-2K small ones
    # (DMA here is descriptor-overhead bound, not byte-rate bound).
    xTp = nc.declare_dram_parameter("xTp", [128, ND * T], MM, isOutput=False)
    wpk = nc.declare_dram_parameter("wpk", [128, WPK], MM, isOutput=False)
    qkb = nc.declare_dram_parameter("qkb", [128, 1], F32, isOutput=False)
    out = nc.declare_dram_parameter("out", [T, HS], F32, isOutput=True)

    scale = float(1.0 / np.sqrt(HS))

    with tile.TileContext(nc) as tc:
        with tc.tile_pool(name="const", bufs=1) as cpool, \
             tc.tile_pool(name="big", bufs=1) as bpool, \
             tc.tile_pool(name="pex", bufs=28) as ppool, \
             tc.tile_pool(name="osb", bufs=2) as opool, \
             tc.tile_pool(name="fin", bufs=3) as fpool:

            # ---- all weights/biases in one packed DMA on the SWDGE queue,
            # leaving both HWDGE rings free for the x.T slabs
            wpk_t = cpool.tile([128, WPK], MM, tag="wpk")
            nc.gpsimd.dma_start(wpk_t[:], wpk[:])
            wqk_t = wpk_t[:, 0:ND * 2 * HS]                  # [128, ND*128]
            wv_t = wpk_t[:, ND * 2 * HS:ND * 2 * HS + ND * HS]
            vbB_t = wpk_t[:, WPK - HS - 1:WPK - 1]           # [128, HS]
            qkb_t = cpool.tile([128, 1], F32, tag="qkb")
            nc.gpsimd.dma_start(qkb_t[:], qkb[:])

            # ---- x.T as 8 per-d-chunk DMAs (4KB contiguous per partition on
            # both ends -> large descriptors), round-robin on the two fair
            # HWDGE rings so chunks land staggered and the first projection
            # can consume them as they arrive
            xTs = bpool.tile([128, ND, T], MM, tag="xTs")
            for dc in range(ND):
                (nc.sync, nc.scalar, nc.gpsimd)[dc % 3].dma_start(
                    xTs[:, dc, :], xTp[:, dc * T:(dc + 1) * T])

            # ---- constants ----
            id_32 = cpool.tile([128, 128], F32, tag="id_32")
            make_identity(nc, id_32[:])
            # 0/1 lower-causal mask for the 128-wide diagonal sub-block of
            # S.T chunks (keep iff f >= p); built f32 (POOL), applied in MM
            trimask = cpool.tile([128, 128], F32, tag="trimask")
            nc.gpsimd.memset(trimask[:], 1.0)
            nc.gpsimd.affine_select(
                out=trimask[:], in_=trimask[:],
                compare_op=mybir.AluOpType.is_ge,
                fill=0.0, base=0,
                pattern=[[1, 128]], channel_multiplier=-1)
            if MM is F32:
                id_mm = id_32
                tm_mm = trimask
            else:
                id_mm = cpool.tile([128, 128], MM, tag="id_mm")
                nc.vector.tensor_copy(id_mm[:], id_32[:])
                tm_mm = cpool.tile([128, 128], MM, tag="tm_mm")
                nc.vector.tensor_copy(tm_mm[:], trimask[:])

            QT = bpool.tile([64, T], MM, tag="QT")
            KT = bpool.tile([64, T], MM, tag="KT")
            VTr = bpool.tile([64, T], MM, tag="VTr")
            Vn = bpool.tile([128, NKC, HS + 1], MM, tag="Vn")
            ones16 = cpool.tile([128, NKC, 1], F32, tag="ones16")
            nc.gpsimd.memset(ones16[:], 1.0)
            nc.vector.tensor_copy(Vn[:, :, HS:HS + 1], ones16[:])

            with tc.tile_pool(name="psS", bufs=4, space="PSUM") as psS, \
                 tc.tile_pool(name="psO", bufs=2, space="PSUM") as psO, \
                 tc.tile_pool(name="psT", bufs=2, space="PSUM") as psT:

                # warm the PE (HAM clock gate) with throwaway transposes of
                # the identity while the x.T slabs land, so the projections
                # start at full clock
                wu = psT.tile([128, 128], MM, tag="pt")
                for _ in range(40):
                    nc.tensor.transpose(wu[:], id_mm[:], id_mm[:])

                def stage_gen(j):
                    """Projections + V naturalization + S matmuls/exps for
                    query block j.  Yields between small instruction groups
                    so the caller can weave this work between the O-matmuls
                    and finalize of block j-1 (PE stays busy while ScalarE
                    works through the previous block's exps, and the exp
                    stream runs a full block ahead of the O accumulation)."""
                    sl = slice(j * QB, (j + 1) * QB)
                    ps = psS.tile([128, QB], F32, tag="spsum")
                    for dc in range(ND):
                        nc.tensor.matmul(ps[:], wqk_t[:, dc * 128:(dc + 1) * 128],
                                         xTs[:, dc, sl],
                                         start=(dc == 0), stop=(dc == ND - 1))
                        if dc % 3 == 2:
                            yield None
                    nc.vector.tensor_scalar_add(QT[:, sl], ps[0:64, :],
                                                qkb_t[0:64, :])
                    nc.vector.tensor_scalar_add(KT[:, sl], ps[64:128, :],
                                                qkb_t[64:128, :])
                    yield None
                    pv = psS.tile([128, QB], F32, tag="spsum")
                    for dc in range(ND):
                        nc.tensor.matmul(pv[0:64, :],
                                         wv_t[:, dc * HS:(dc + 1) * HS],
                                         xTs[:, dc, sl],
                                         start=(dc == 0), stop=(dc == ND - 1))
                        if dc % 3 == 2:
                            yield None
                    nc.scalar.copy(VTr[:, sl], pv[0:64, :])
                    yield None
                    for c in range(4 * j, 4 * j + 4):
                        pt = psT.tile([128, HS], MM, tag="pt")
                        nc.tensor.transpose(pt[:], VTr[:, c * 128:(c + 1) * 128],
                                            id_mm[0:64, 0:64])
                        nc.vector.tensor_add(Vn[:, c, 0:HS], pt[:], vbB_t[:])
                        yield None
                    pexp = []
                    for c in range(4 * j + 4):
                        r = c - 4 * j
                        f0 = max(0, 128 * r)      # first live column in block
                        qsl = slice(j * QB + f0, (j + 1) * QB)
                        ps = psS.tile([128, QB], F32, tag="spsum")
                        nc.tensor.matmul(ps[:, f0:QB],
                                         KT[:, c * 128:(c + 1) * 128],
                                         QT[:, qsl], start=True, stop=True)
                        pe = ppool.tile([128, QB], MM, tag="pexp")
                        nc.scalar.activation(pe[:, f0:QB], ps[:, f0:QB],
                                             mybir.ActivationFunctionType.Exp,
                                             scale=scale)
                        if r >= 0:
                            # keep S.T[p, f0+f'] iff f' - p >= 0 (tk <= tq);
                            # only a diagonal chunk's first 128 cols need it
                            nc.vector.tensor_mul(pe[:, f0:f0 + 128],
                                                 pe[:, f0:f0 + 128], tm_mm[:])
                        pexp.append((pe, f0))
                        yield pexp

                # block 0 runs eagerly (nothing to weave it into)
                pexp_cur = []
                for v in stage_gen(0):
                    if v is not None:
                        pexp_cur = v

                pexp_by_block = {0: pexp_cur}
                outqs = (nc.sync, nc.scalar)
                for j in range(NQB):
                    if j + 1 < NQB:
                        gen = stage_gen(j + 1)
                        pexp_by_block[j + 1] = []
                    else:
                        gen = None

                    def adv(n=1):
                        nonlocal gen
                        if gen is None:
                            return
                        for _ in range(n):
                            try:
                                v = next(gen)
                                if v is not None:
                                    pexp_by_block[j + 1] = v
                            except StopIteration:
                                gen = None
                                return

                    ncl = 4 * j + 4
                    pexp = pexp_by_block[j]
                    po = psO.tile([128, QB], F32, tag="opsum")
                    for c in range(ncl):
                        pe, f0 = pexp[c]
                        nc.tensor.matmul(po[0:HS + 1, f0:QB], Vn[:, c, :],
                                         pe[:, f0:QB],
                                         start=(c == 0), stop=(c == ncl - 1))
                        adv(2)
                    fin = fpool.tile([128, QB // 128, HS], F32, tag="fin")
                    for tt in range(QB // 128):
                        ob = opool.tile([HS + 1, 128], F32, tag="ob")
                        nc.vector.tensor_copy(ob[:],
                                              po[0:HS + 1, tt * 128:(tt + 1) * 128])
                        pt2 = psT.tile([128, HS + 1], F32, tag="pt")
                        nc.tensor.transpose(pt2[:], ob[:],
                                            id_32[0:HS + 1, 0:HS + 1])
                        rc = fpool.tile([128, 1], F32, tag="rc")
                        nc.vector.reciprocal(rc[:], pt2[:, HS:HS + 1])
                        nc.vector.tensor_scalar_mul(fin[:, tt, :],
                                                    pt2[:, 0:HS], rc[:])
                        adv(2)
                        r0 = j * QB + tt * 128
                        outqs[(4 * j + tt) % 2].dma_start(out[r0:r0 + 128, :],
                                                          fin[:, tt, :])
                    while gen is not None:
                        adv(1)

    nc.compile()
    return nc


_RUNNERS = {}


def _get_runner(mode=None):
    mode = mode or MM_MODE
    if mode not in _RUNNERS:
        _RUNNERS[mode] = build(mode)
    return _RUNNERS[mode]


def _host_dt(mode=None):
    if (mode or MM_MODE) == "bf16":
        import ml_dtypes
        return ml_dtypes.bfloat16
    return np.float32


def make_in_maps(x, wq_w, wq_b, wk_w, wk_b, wv_w, wv_b, mode=None):
    hd = _host_dt(mode)
    x = np.asarray(x, np.float32)
    # packed per-partition weight image: [wqk chunks | wv chunks | vb | qkb]
    wqk = np.concatenate([np.asarray(wq_w, np.float32),
                          np.asarray(wk_w, np.float32)], axis=0)  # [128, D]
    wv = np.asarray(wv_w, np.float32)                             # [64, D]
    wpk = np.zeros((128, WPK), np.float32)
    for dc in range(ND):
        wpk[:, dc * 128:(dc + 1) * 128] = wqk[:, dc * 128:(dc + 1) * 128].T
        wpk[0:128, ND * 128 + dc * HS:ND * 128 + (dc + 1) * HS] = \
            wv[:, dc * 128:(dc + 1) * 128].T
    wpk[:, WPK - HS - 1:WPK - 1] = np.broadcast_to(
        np.asarray(wv_b, np.float32), (128, HS))
    wpk[:, WPK - 1] = np.concatenate([np.asarray(wq_b, np.float32),
                                      np.asarray(wk_b, np.float32)])
    wpk = np.ascontiguousarray(wpk).astype(hd)
    qkb = np.concatenate([np.asarray(wq_b, np.float32),
                          np.asarray(wk_b, np.float32)])[:, None].copy()
    in_maps = []
    for b in range(B):
        xTp = np.ascontiguousarray(
            x[b].T.reshape(ND, 128, T).transpose(1, 0, 2).reshape(128, ND * T)
        ).astype(hd)
        in_maps.append({"xTp": xTp, "wpk": wpk, "qkb": qkb})
    return in_maps


def run(in_maps, trace=False, tmpdir=None, mode=None):
    nc = _get_runner(mode)
    return run_bass_kernel_spmd(nc, in_maps, core_ids=list(range(NCORES)),
                                trace=trace, tmpdir=tmpdir)


def _canary_ok(out, x, wq_w, wq_b, wk_w, wk_b, wv_w, wv_b):
    """Cheap exact check of causal rows t=0,1 (closed-form, tiny host cost).

    Catches catastrophic HW-mode failures (zeros/garbage) while passing
    reduced-precision rounding. Row 0 attends only key 0 -> out = v[0];
    row 1 is a two-term softmax.
    """
    x2 = np.asarray(x, np.float32)[:, 0:2, :].astype(np.float64)      # [B,2,D]
    q = x2 @ np.asarray(wq_w, np.float64).T + np.asarray(wq_b, np.float64)
    k = x2 @ np.asarray(wk_w, np.float64).T + np.asarray(wk_b, np.float64)
    v = x2 @ np.asarray(wv_w, np.float64).T + np.asarray(wv_b, np.float64)
    exp0 = v[:, 0, :]                                                 # [B,HS]
    s = np.einsum("bh,bsh->bs", q[:, 1, :], k) / np.sqrt(HS)          # [B,2]
    w = np.exp(s - s.max(-1, keepdims=True))
    w = w / w.sum(-1, keepdims=True)
    exp1 = np.einsum("bs,bsh->bh", w, v)
    got = np.stack([out[:, 0, :], out[:, 1, :]], axis=1)
    want = np.stack([exp0, exp1], axis=1)
    rel = np.abs(got - want) / max(np.abs(want).max(), 1e-6)
    return np.isfinite(got).all() and rel.max() < 3e-2


def kernel(x, wq_w, wq_b, wk_w, wk_b, wv_w, wv_b):
    args = (x, wq_w, wq_b, wk_w, wk_b, wv_w, wv_b)
    res = run(make_in_maps(*args, mode=MM_MODE), mode=MM_MODE)
    out = np.stack([np.asarray(res.results[b]["out"], np.float32)
                    for b in range(B)], axis=0)
    if MM_MODE != FALLBACK_MODE and not _canary_ok(out, *args):
        # fast matmul mode produced bad numerics on this HW; fall back to
        # the plain-fp32 kernel
        res = run(make_in_maps(*args, mode=FALLBACK_MODE), mode=FALLBACK_MODE)
        out = np.stack([np.asarray(res.results[b]["out"], np.float32)
                        for b in range(B)], axis=0)
    return out
